# revision 14
# baseline (speedup 1.0000x reference)
import sys

sys.path.insert(0, "/opt/trn_rl_repo")

import numpy as np
import ml_dtypes

import concourse.bass as bass
import concourse.tile as tile
from concourse import bacc, mybir
from concourse.bass_utils import run_bass_kernel_spmd

# Problem constants (hardcoded per contract)
B, N, F = 8, 512, 16
D, PH, PW = 150, 26, 26
IMG = 128
HP = IMG + 2 * PH  # 180 padded canvas rows
WP = IMG + 2 * PW  # 180 padded canvas cols
CSTRIDE = 184  # canvas row stride in SBUF (padded)
HW = PH * PW  # 676
C = 64  # spline coefficients per voxel
GRP = 128  # groups per core
EPG = 4  # emitters per group
EW = 32  # partition rows per emitter (F=16 + 16 zero pad, 32-aligned)
K = EPG * C  # 256 contraction (block diagonal)
KC = K // 128  # 2 K-chunks

_compiled = None


def _build_bass():
    nc = bacc.Bacc()
    f32 = mybir.dt.float32
    bf16 = mybir.dt.bfloat16
    i32 = mybir.dt.int32

    lhsT_d = nc.declare_dram_parameter("lhsT", [GRP, K, 128], bf16, isOutput=False)
    rhs_d = nc.declare_dram_parameter("rhs", [GRP, K, HW], bf16, isOutput=False)
    offs_d = nc.declare_dram_parameter("offs", [1, N], i32, isOutput=False)
    out_d = nc.declare_dram_parameter("out", [F, IMG * IMG], f32, isOutput=True)

    with tile.TileContext(nc) as tc:
        with (
            tc.tile_pool(name="canvas", bufs=1) as canvas_pool,
            tc.tile_pool(name="weights", bufs=3) as w_pool,
            tc.tile_pool(name="slabs", bufs=3) as s_pool,
            tc.tile_pool(name="psum", bufs=3, space="PSUM") as p_pool,
            tc.tile_pool(name="small", bufs=1) as small_pool,
        ):
            canvas = canvas_pool.tile([EW, CSTRIDE * CSTRIDE], f32)
            nc.gpsimd.memset(canvas[:], 0.0)
            canvas3 = canvas[:].rearrange(
                "p (h w) -> p h w", h=CSTRIDE, w=CSTRIDE
            )

            offs_t = small_pool.tile([1, N], i32)
            nc.sync.dma_start(offs_t[:], offs_d[:])
            off_reg = nc.vector.alloc_register("off_reg")

            for g in range(GRP):
                lt = w_pool.tile([128, KC * 128], bf16, tag="lt")
                nc.sync.dma_start(
                    lt[:].rearrange("p (kc m) -> p kc m", kc=KC),
                    lhsT_d[g].rearrange("(kc k) m -> k kc m", k=128),
                )
                rt = s_pool.tile([128, KC * HW], bf16, tag="rt")
                nc.sync.dma_start(
                    rt[:].rearrange("p (kc n) -> p kc n", kc=KC),
                    rhs_d[g].rearrange("(kc k) n -> k kc n", k=128),
                )
                ps = p_pool.tile([128, HW], f32, tag="ps")
                for kc in range(KC):
                    for n0, n1 in ((0, 512), (512, HW)):
                        nc.tensor.matmul(
                            ps[:, n0:n1],
                            lhsT=lt[:, kc * 128 : (kc + 1) * 128],
                            rhs=rt[:, kc * HW + n0 : kc * HW + n1],
                            start=(kc == 0),
                            stop=(kc == KC - 1),
                        )
                sbp = s_pool.tile([128, HW], f32, tag="sbp")
                nc.scalar.copy(out=sbp[:], in_=ps[:])
                ps3 = sbp[:].rearrange("p (h w) -> p h w", h=PH, w=PW)
                for i in range(EPG):
                    e = g * EPG + i
                    nc.vector.reg_load(off_reg, offs_t[0:1, e : e + 1])
                    off = nc.vector.snap(
                        off_reg,
                        donate=True,
                        min_val=0,
                        max_val=(HP - PH) * CSTRIDE + (WP - PW),
                    )
                    dst = canvas[:, bass.ds(off, (PH - 1) * CSTRIDE + PW)]
                    dst = bass.AP(
                        dst.tensor,
                        dst.offset,
                        [dst.ap[0], [CSTRIDE, PH], [1, PW]],
                    )
                    nc.vector.tensor_tensor(
                        out=dst,
                        in0=dst,
                        in1=ps3[EW * i : EW * (i + 1)],
                        op=mybir.AluOpType.add,
                    )

            # crop canvas -> out
            nc.sync.dma_start(
                out_d[:].rearrange("p (h w) -> p h w", h=IMG, w=IMG),
                canvas3[0:F, PH : PH + IMG, PW : PW + IMG],
            )
    if not nc.is_finalized():
        nc.finalize()
    return nc


def _host_prep(xyz, n_photons, coeffs, inv_voxel_size, psf_center):
    """Per-batch host prep: indices, series, photon-folded lhsT, gathered rhs."""
    u = xyz * inv_voxel_size  # (B,N,3)
    u = u.copy()
    u[..., :2] -= psf_center[:2]
    u[..., 2] += psf_center[2]
    u_floor = np.floor(u)
    frac = u - u_floor
    ui = u_floor.astype(np.int32)
    x_idx = ui[..., 0] + PW  # (B,N)
    y_idx = ui[..., 1] + PH
    z_idx = ui[..., 2]
    frac[..., :2] = 1.0 - frac[..., :2]

    # 64-term series: series[b,n,c], c = kz*16 + kx*4 + ky
    p = frac[..., None] ** np.arange(4, dtype=np.float32)  # (B,N,3,4)
    vx, vy, vz = p[..., 0, :], p[..., 1, :], p[..., 2, :]
    series = (
        vz[..., :, None, None] * vx[..., None, :, None] * vy[..., None, None, :]
    ).reshape(B, N, C)

    # photon folding: series16[b,n,f,c]
    series16 = n_photons[..., None] * series[:, :, None, :]  # (B,N,F,C)

    # lhsT[b,g,(slot,c),(i,f)] block diagonal; each emitter owns a 32-col
    # block (16 frames + 16 zero cols) so psum slices are 32-aligned.
    lhsT = np.zeros((B, GRP, K, 128), dtype=np.float32)
    s16g = series16.reshape(B, GRP, EPG, F, C)
    for i in range(EPG):
        lhsT[:, :, i * C : (i + 1) * C, i * EW : i * EW + F] = s16g[
            :, :, i
        ].transpose(0, 1, 3, 2)
    lhsT = lhsT.astype(ml_dtypes.bfloat16)

    # rhs: gathered transposed slabs (c, hw) per emitter
    coeffs_t = np.ascontiguousarray(
        coeffs.reshape(D, HW, C).transpose(0, 2, 1)
    ).astype(ml_dtypes.bfloat16)  # (D, C, HW)
    rhs = coeffs_t[z_idx.reshape(-1)].reshape(B, GRP, K, HW)

    offs = (y_idx * CSTRIDE + x_idx).astype(np.int32)  # (B,N)
    return lhsT, rhs, offs


def kernel(xyz, n_photons, coeffs, inv_voxel_size, psf_center, img_size):
    global _compiled
    xyz = np.asarray(xyz, dtype=np.float32)
    n_photons = np.asarray(n_photons, dtype=np.float32)
    coeffs = np.asarray(coeffs, dtype=np.float32)
    inv_voxel_size = np.asarray(inv_voxel_size, dtype=np.float32)
    psf_center = np.asarray(psf_center, dtype=np.float32)

    lhsT, rhs, offs = _host_prep(xyz, n_photons, coeffs, inv_voxel_size, psf_center)

    if _compiled is None:
        _compiled = _build_bass()
    nc = _compiled

    in_maps = [
        {"lhsT": lhsT[b], "rhs": rhs[b], "offs": offs[b : b + 1]}
        for b in range(B)
    ]
    res = run_bass_kernel_spmd(nc, in_maps, core_ids=list(range(B)))
    out = np.stack(
        [res.results[b]["out"].reshape(F, IMG, IMG) for b in range(B)], axis=0
    )
    return out
